# revision 6
# baseline (speedup 1.0000x reference)
"""Trainium2 Bass kernel for nn_Attention_84473416778449.

Reference computation (B=2, S=2048, D=1024, H=16, HD=64, fp32):
    q/k/v = x @ w{q,k,v}.T ; RoPE(q, k) ; causal softmax attention ; out @ wo.T

Sharding: 8 cores = (batch 2) x (head-group 4). Each core computes 4 heads of
one batch end-to-end and a partial output projection over its 256 channels;
the host sums the 4 partials per batch.

On-chip layout (per core):
    xT    [1024, 2048]  (host pre-transposed x[b].T), fp32r in SBUF
    wqT/wkT/wvT [1024, 256] (host pre-transposed w[rows,:].T)
    woT   [256, 1024]   (host pre-transposed wo[:, cols].T)
    qT/kT [2 x (128, 2048)] two heads per tile (dh on partitions), RoPE applied
    v     [16 x (128, 4*65)] s on partitions, per-head 64 cols + ones col
          (ones column makes the PV matmul also produce softmax denominators)
    scores^T tiles [sk=128, sq=512] -> exp on ACT -> causal mask via
          affine_select -> PV accumulate -> normalize by reciprocal of row 64.

All matmuls run in float32r (tf32-like, ~1.5e-4 rel err, full PE rate at
moving-dim >= 256). Causal masking is generated on-chip (the reference mask is
the standard causal mask; fully-masked score tiles are skipped entirely).
"""
import sys

if "/opt/trn_rl_repo" not in sys.path:
    sys.path.insert(0, "/opt/trn_rl_repo")

import numpy as np

import concourse.bass as bass
import concourse.mybir as mybir
import concourse.tile as tile
from concourse import bacc
from concourse.bass_utils import run_bass_kernel_spmd

B, S, D, H, HD = 2, 2048, 1024, 16, 64
NCORES = 8
GROUPS = 4            # head groups
GH = H // GROUPS      # heads per group = 4
GC = GH * HD          # channels per group = 256
KT = D // 128         # 8 k-tiles over D
ST = S // 128         # 16 s-tiles
QB = 4                # sq blocks of 512
QW = S // QB          # 512

f32 = mybir.dt.float32
f32r = mybir.dt.float32r
Exp = mybir.ActivationFunctionType.Exp
Copy = mybir.ActivationFunctionType.Copy

_cache = {}


def _build():
    nc = bacc.Bacc("TRN2", num_devices=NCORES)

    xT = nc.dram_tensor("xT", [D, S], f32, kind="ExternalInput").ap()
    wqT = nc.dram_tensor("wqT", [D, GC], f32, kind="ExternalInput").ap()
    wkT = nc.dram_tensor("wkT", [D, GC], f32, kind="ExternalInput").ap()
    wvT = nc.dram_tensor("wvT", [D, GC], f32, kind="ExternalInput").ap()
    woT = nc.dram_tensor("woT", [GC, D], f32, kind="ExternalInput").ap()
    cs2 = nc.dram_tensor("cs2", [128, S], f32, kind="ExternalInput").ap()
    sn2 = nc.dram_tensor("sn2", [128, S], f32, kind="ExternalInput").ap()
    out = nc.dram_tensor("out", [S, D], f32, kind="ExternalOutput").ap()

    with tile.TileContext(nc) as tc:
        with tc.tile_pool(name="persist", bufs=1) as pp, \
             tc.tile_pool(name="rope", bufs=2) as rp, \
             tc.tile_pool(name="probs", bufs=3) as wp, \
             tc.tile_pool(name="outsb", bufs=3) as op_, \
             tc.tile_pool(name="small", bufs=2) as sp:

            # ---- constants (built in fp32, cast to fp32r via DVE copy) -----
            cscratch = pp.tile([128, 128], f32, tag="cscratch")
            nc.gpsimd.memset(cscratch[:], 0.0)
            for blk in range(2):
                sub = cscratch[blk * 64:(blk + 1) * 64,
                               blk * 64:(blk + 1) * 64]
                # -1 where p - f == 32  (covers f<32 inside a 64x64 block)
                nc.gpsimd.affine_select(
                    out=sub, in_=sub, pattern=[[-1, 64]], base=-32,
                    channel_multiplier=1,
                    compare_op=mybir.AluOpType.not_equal, fill=-1.0)
                # +1 where f - p == 32
                nc.gpsimd.affine_select(
                    out=sub, in_=sub, pattern=[[1, 64]], base=-32,
                    channel_multiplier=-1,
                    compare_op=mybir.AluOpType.not_equal, fill=1.0)
            rt2 = pp.tile([128, 128], f32r, tag="rt2")
            nc.vector.tensor_copy(rt2[:], cscratch[:])
            cscratch2 = pp.tile([128, 128], f32, tag="cscratch2")
            nc.gpsimd.memset(cscratch2[:], 0.0)
            nc.gpsimd.affine_select(
                out=cscratch2[:], in_=cscratch2[:], pattern=[[-1, 128]],
                base=0, channel_multiplier=1,
                compare_op=mybir.AluOpType.not_equal, fill=1.0)
            ident = pp.tile([128, 128], f32r, tag="ident")
            nc.vector.tensor_copy(ident[:], cscratch2[:])
            ones_f = pp.tile([128, GH], f32, tag="ones_f")
            nc.gpsimd.memset(ones_f[:], 1.0)

            # ---- loads -----------------------------------------------------
            xTr = []
            for kt in range(KT):
                t = pp.tile([128, S], f32r, tag=f"xT{kt}")
                nc.gpsimd.dma_start(t[:], xT[kt * 128:(kt + 1) * 128, :])
                xTr.append(t)

            def load_wT(src):
                t = pp.tile([128, KT * GC], f32r, tag=f"w{src.tensor.name}")
                nc.gpsimd.dma_start(
                    t[:].rearrange("p (t c) -> p t c", c=GC),
                    src.rearrange("(t p) c -> p t c", p=128))
                return t

            wq_s, wk_s, wv_s = load_wT(wqT), load_wT(wkT), load_wT(wvT)
            wo_s = []
            for kt in range(2):
                t = pp.tile([128, D], f32r, tag=f"wo{kt}")
                nc.gpsimd.dma_start(t[:], woT[kt * 128:(kt + 1) * 128, :])
                wo_s.append(t)
            cs_sb = pp.tile([128, S], f32, tag="cs")
            nc.sync.dma_start(cs_sb[:], cs2[:])
            sn_sb = pp.tile([128, S], f32, tag="sn")
            nc.sync.dma_start(sn_sb[:], sn2[:])

            qT = [pp.tile([128, S], f32r, tag=f"qT{i}", name=f"qT{i}") for i in range(2)]
            kTt = [pp.tile([128, S], f32r, tag=f"kT{i}", name=f"kT{i}") for i in range(2)]
            attnT = [pp.tile([128, S], f32r, tag=f"aT{i}", name=f"aT{i}") for i in range(2)]
            v_sb = [pp.tile([128, GH * (HD + 1)], f32r, tag=f"v{i}",
                             name=f"v{i}") for i in range(ST)]

            # ---- phase A: q/k projections + RoPE --------------------------
            with tc.tile_pool(name="psA", bufs=3, space="PSUM") as psA, \
                 tc.tile_pool(name="psR", bufs=2, space="PSUM") as psR:
                for w_src, dst in ((wq_s, qT), (wk_s, kTt)):
                    for hp in range(2):
                        for sb_i in range(QB):
                            pq = psA.tile([128, QW], f32, tag="proj")
                            for kt in range(KT):
                                nc.tensor.matmul(
                                    pq[:],
                                    w_src[:, kt * GC + hp * 128:
                                          kt * GC + hp * 128 + 128],
                                    xTr[kt][:, sb_i * QW:(sb_i + 1) * QW],
                                    start=(kt == 0), stop=(kt == KT - 1))
                            tcs = rp.tile([128, QW], f32r, tag="tcs")
                            nc.vector.tensor_tensor(
                                out=tcs[:], in0=pq[:],
                                in1=cs_sb[:, sb_i * QW:(sb_i + 1) * QW],
                                op=mybir.AluOpType.mult)
                            tsn = rp.tile([128, QW], f32r, tag="tsn")
                            nc.vector.tensor_tensor(
                                out=tsn[:], in0=pq[:],
                                in1=sn_sb[:, sb_i * QW:(sb_i + 1) * QW],
                                op=mybir.AluOpType.mult)
                            pr = psR.tile([128, QW], f32, tag="rope")
                            nc.tensor.matmul(pr[:], rt2[:], tsn[:],
                                             start=True, stop=False)
                            nc.tensor.matmul(pr[:], ident[:], tcs[:],
                                             start=False, stop=True)
                            nc.scalar.activation(
                                dst[hp][:, sb_i * QW:(sb_i + 1) * QW],
                                pr[:], Copy)

                # ---- v projection (natural layout, ones-augmented) --------
                with tc.tile_pool(name="psV", bufs=2, space="PSUM") as psV:
                    for st in range(ST):
                        pv = psV.tile([128, GC], f32, tag="vproj")
                        for kt in range(KT):
                            nc.tensor.matmul(
                                pv[:],
                                xTr[kt][:, st * 128:(st + 1) * 128],
                                wv_s[:, kt * GC:(kt + 1) * GC],
                                start=(kt == 0), stop=(kt == KT - 1))
                        vt = v_sb[st]
                        nc.vector.tensor_copy(
                            vt[:].rearrange("p (h e) -> p h e", e=HD + 1)
                            [:, :, HD:HD + 1], ones_f[:])
                        nc.scalar.activation(
                            vt[:].rearrange("p (h e) -> p h e", e=HD + 1)
                            [:, :, 0:HD],
                            pv[:].rearrange("p (h d) -> p h d", d=HD),
                            Copy)

            # ---- phase B: attention per head ------------------------------
            with tc.tile_pool(name="psS", bufs=4, space="PSUM") as psS, \
                 tc.tile_pool(name="psO", bufs=2, space="PSUM") as psO:
                for h in range(GH):
                    hp, pb = h // 2, (h % 2) * 64
                    for qb in range(QB):
                        po = psO.tile([HD + 1, QW], f32, tag="pvacc")
                        nsk = (qb + 1) * (QW // 128)
                        for kt in range(nsk):
                            pscore = psS.tile([128, QW], f32, tag="score")
                            nc.tensor.matmul(
                                pscore[:],
                                kTt[hp][pb:pb + 64, kt * 128:(kt + 1) * 128],
                                qT[hp][pb:pb + 64, qb * QW:(qb + 1) * QW],
                                start=True, stop=True)
                            prt = wp.tile([128, QW], f32r, tag="probs")
                            nc.scalar.activation(prt[:], pscore[:], Exp,
                                                 scale=0.125)
                            if kt >= nsk - 4:
                                nc.gpsimd.affine_select(
                                    out=prt[:], in_=prt[:],
                                    pattern=[[1, QW]],
                                    base=qb * QW - kt * 128,
                                    channel_multiplier=-1,
                                    compare_op=mybir.AluOpType.is_ge,
                                    fill=0.0)
                            nc.tensor.matmul(
                                po[:],
                                v_sb[kt][:, h * (HD + 1):(h + 1) * (HD + 1)],
                                prt[:],
                                start=(kt == 0), stop=(kt == nsk - 1))
                        rc = sp.tile([1, QW], f32, tag="recip")
                        nc.vector.reciprocal(rc[:], po[HD:HD + 1, :])
                        bc = sp.tile([64, QW], f32, tag="bcast")
                        nc.gpsimd.partition_broadcast(bc[:], rc[:])
                        nc.vector.tensor_tensor(
                            out=attnT[hp][pb:pb + 64, qb * QW:(qb + 1) * QW],
                            in0=po[0:HD, :], in1=bc[:],
                            op=mybir.AluOpType.mult)

            # ---- phase C: output projection -------------------------------
            with tc.tile_pool(name="psC", bufs=3, space="PSUM") as psC:
                for st in range(ST):
                    for db in range(2):
                        pc = psC.tile([128, QW], f32, tag="oproj")
                        for kt in range(2):
                            nc.tensor.matmul(
                                pc[:],
                                attnT[kt][:, st * 128:(st + 1) * 128],
                                wo_s[kt][:, db * QW:(db + 1) * QW],
                                start=(kt == 0), stop=(kt == 1))
                        ob = op_.tile([128, QW], f32, tag="outsb")
                        nc.scalar.activation(ob[:], pc[:], Copy)
                        nc.sync.dma_start(
                            out[st * 128:(st + 1) * 128,
                                db * QW:(db + 1) * QW], ob[:])

    nc.compile()
    return nc


def _shard_inputs(x, cos, sin, wq, wk, wv, wo):
    x = np.ascontiguousarray(x, dtype=np.float32)
    cosT = np.ascontiguousarray(cos.reshape(S, HD).T, dtype=np.float32)
    sinT = np.ascontiguousarray(sin.reshape(S, HD).T, dtype=np.float32)
    cs2 = np.ascontiguousarray(np.concatenate([cosT, cosT], axis=0))
    sn2 = np.ascontiguousarray(np.concatenate([sinT, sinT], axis=0))
    in_maps = []
    for c in range(NCORES):
        b, g = c // GROUPS, c % GROUPS
        rows = slice(g * GC, (g + 1) * GC)
        in_maps.append({
            "xT": np.ascontiguousarray(x[b].T),
            "wqT": np.ascontiguousarray(np.asarray(wq, np.float32)[rows, :].T),
            "wkT": np.ascontiguousarray(np.asarray(wk, np.float32)[rows, :].T),
            "wvT": np.ascontiguousarray(np.asarray(wv, np.float32)[rows, :].T),
            "woT": np.ascontiguousarray(np.asarray(wo, np.float32)[:, rows].T),
            "cs2": cs2,
            "sn2": sn2,
        })
    return in_maps


def _run(inputs, trace=False, trace_kwargs=None):
    if "nc" not in _cache:
        _cache["nc"] = _build()
    nc = _cache["nc"]
    in_maps = _shard_inputs(
        inputs["x"], inputs["cos"], inputs["sin"],
        inputs["wq"], inputs["wk"], inputs["wv"], inputs["wo"])
    res = run_bass_kernel_spmd(
        nc, in_maps, list(range(NCORES)), trace=trace,
        **(trace_kwargs or {}))
    full = np.zeros((B, S, D), dtype=np.float32)
    for c in range(NCORES):
        full[c // GROUPS] += res.results[c]["out"]
    return full, res


def kernel(**inputs):
    full, _ = _run(inputs, trace=False)
    return full


# revision 8
# speedup vs baseline: 1.1226x; 1.1226x over previous
"""Trainium2 Bass kernel for nn_Attention_84473416778449.

Reference computation (B=2, S=2048, D=1024, H=16, HD=64, fp32):
    q/k/v = x @ w{q,k,v}.T ; RoPE(q, k) ; causal softmax attention ; out @ wo.T

Sharding: 8 cores = (batch 2) x (head-group 4). Each core computes 4 heads of
one batch end-to-end and a partial output projection over its 256 channels;
the host sums the 4 partials per batch.

On-chip layout (per core):
    xT    [1024, 2048]  (host pre-transposed x[b].T)
    wqT/wkT/wvT [1024, 256] (host pre-transposed w[rows,:].T)
    woT   [256, 1024]   (host pre-transposed wo[:, cols].T)
    qT/kT [2 x (128, 2048)] two heads per tile (dh on partitions), RoPE applied
    v     [16 x (128, 4*65)] s on partitions, per-head 64 cols + ones col
          (ones column makes the PV matmul also produce softmax denominators)
    scores^T tiles [sk=128, sq=512] -> exp on ACT -> causal mask via
          affine_select -> PV accumulate -> normalize by reciprocal of row 64.

Matmul operands are fp16 (full PE rate + fast weight load; ~5e-4 rounding,
4 extra mantissa bits vs bf16); accumulation is always fp32 in PSUM. Causal
masking is generated on-chip; fully-masked score tiles are skipped entirely.
Paired heads live at partitions 0-63 / 64-127 of one tile, so their K=64
QK matmuls target different PE row-groups and run concurrently.
"""
import sys

if "/opt/trn_rl_repo" not in sys.path:
    sys.path.insert(0, "/opt/trn_rl_repo")

import numpy as np

import concourse.bass as bass
import concourse.mybir as mybir
import concourse.tile as tile
from concourse import bacc
from concourse.bass_utils import run_bass_kernel_spmd

B, S, D, H, HD = 2, 2048, 1024, 16, 64
NCORES = 8
GROUPS = 4            # head groups
GH = H // GROUPS      # heads per group = 4
GC = GH * HD          # channels per group = 256
KT = D // 128         # 8 k-tiles over D
ST = S // 128         # 16 s-tiles
QB = 4                # sq blocks of 512
QW = S // QB          # 512

f32 = mybir.dt.float32
MMDT = mybir.dt.float16   # matmul-operand dtype
Exp = mybir.ActivationFunctionType.Exp
Copy = mybir.ActivationFunctionType.Copy

_cache = {}


def _build():
    nc = bacc.Bacc("TRN2", num_devices=NCORES)

    xT = nc.dram_tensor("xT", [D, S], f32, kind="ExternalInput").ap()
    wqT = nc.dram_tensor("wqT", [D, GC], f32, kind="ExternalInput").ap()
    wkT = nc.dram_tensor("wkT", [D, GC], f32, kind="ExternalInput").ap()
    wvT = nc.dram_tensor("wvT", [D, GC], f32, kind="ExternalInput").ap()
    woT = nc.dram_tensor("woT", [GC, D], f32, kind="ExternalInput").ap()
    cs2 = nc.dram_tensor("cs2", [128, S], f32, kind="ExternalInput").ap()
    sn2 = nc.dram_tensor("sn2", [128, S], f32, kind="ExternalInput").ap()
    out = nc.dram_tensor("out", [S, D], f32, kind="ExternalOutput").ap()

    with tile.TileContext(nc) as tc:
        with tc.tile_pool(name="persist", bufs=1) as pp, \
             tc.tile_pool(name="rope", bufs=3) as rp, \
             tc.tile_pool(name="probs", bufs=6) as wp, \
             tc.tile_pool(name="outsb", bufs=3) as op_, \
             tc.tile_pool(name="small", bufs=3) as sp:

            # ---- constants (built in fp32, cast via DVE copy) --------------
            cscratch = pp.tile([128, 128], f32, tag="cscratch")
            nc.gpsimd.memset(cscratch[:], 0.0)
            for blk in range(2):
                sub = cscratch[blk * 64:(blk + 1) * 64,
                               blk * 64:(blk + 1) * 64]
                # -1 where p - f == 32  (covers f<32 inside a 64x64 block)
                nc.gpsimd.affine_select(
                    out=sub, in_=sub, pattern=[[-1, 64]], base=-32,
                    channel_multiplier=1,
                    compare_op=mybir.AluOpType.not_equal, fill=-1.0)
                # +1 where f - p == 32
                nc.gpsimd.affine_select(
                    out=sub, in_=sub, pattern=[[1, 64]], base=-32,
                    channel_multiplier=-1,
                    compare_op=mybir.AluOpType.not_equal, fill=1.0)
            rt2 = pp.tile([128, 128], MMDT, tag="rt2")
            nc.vector.tensor_copy(rt2[:], cscratch[:])
            cscratch2 = pp.tile([128, 128], f32, tag="cscratch2")
            nc.gpsimd.memset(cscratch2[:], 0.0)
            nc.gpsimd.affine_select(
                out=cscratch2[:], in_=cscratch2[:], pattern=[[-1, 128]],
                base=0, channel_multiplier=1,
                compare_op=mybir.AluOpType.not_equal, fill=1.0)
            ident = pp.tile([128, 128], MMDT, tag="ident")
            nc.vector.tensor_copy(ident[:], cscratch2[:])
            ones_f = pp.tile([128, GH], f32, tag="ones_f")
            nc.gpsimd.memset(ones_f[:], 1.0)

            # ---- loads (gpsimd SWDGE casts fp32 -> fp16 in flight) ---------
            xTr = []
            for kt in range(KT):
                t = pp.tile([128, S], MMDT, tag=f"xT{kt}", name=f"xTr{kt}")
                nc.gpsimd.dma_start(t[:], xT[kt * 128:(kt + 1) * 128, :])
                xTr.append(t)

            def load_wT(src):
                t = pp.tile([128, KT * GC], MMDT, tag=f"w{src.tensor.name}",
                            name=f"w{src.tensor.name}")
                nc.gpsimd.dma_start(
                    t[:].rearrange("p (t c) -> p t c", c=GC),
                    src.rearrange("(t p) c -> p t c", p=128))
                return t

            wq_s, wk_s, wv_s = load_wT(wqT), load_wT(wkT), load_wT(wvT)
            wo_s = []
            for kt in range(2):
                t = pp.tile([128, D], MMDT, tag=f"wo{kt}", name=f"wo{kt}")
                nc.gpsimd.dma_start(t[:], woT[kt * 128:(kt + 1) * 128, :])
                wo_s.append(t)
            cs_sb = pp.tile([128, S], f32, tag="cs")
            nc.sync.dma_start(cs_sb[:], cs2[:])
            sn_sb = pp.tile([128, S], f32, tag="sn")
            nc.sync.dma_start(sn_sb[:], sn2[:])

            qT = [pp.tile([128, S], MMDT, tag=f"qT{i}", name=f"qT{i}")
                  for i in range(2)]
            kTt = [pp.tile([128, S], MMDT, tag=f"kT{i}", name=f"kT{i}")
                   for i in range(2)]
            attnT = [pp.tile([128, S], MMDT, tag=f"aT{i}", name=f"aT{i}")
                     for i in range(2)]
            v_sb = [pp.tile([128, GH * (HD + 1)], MMDT, tag=f"v{i}",
                            name=f"v{i}") for i in range(ST)]

            # ---- phase A: q/k projections + RoPE --------------------------
            with tc.tile_pool(name="psA", bufs=3, space="PSUM") as psA, \
                 tc.tile_pool(name="psR", bufs=2, space="PSUM") as psR:
                for w_src, dst in ((wq_s, qT), (wk_s, kTt)):
                    for hp in range(2):
                        for sb_i in range(QB):
                            pq = psA.tile([128, QW], f32, tag="proj")
                            for kt in range(KT):
                                nc.tensor.matmul(
                                    pq[:],
                                    w_src[:, kt * GC + hp * 128:
                                          kt * GC + hp * 128 + 128],
                                    xTr[kt][:, sb_i * QW:(sb_i + 1) * QW],
                                    start=(kt == 0), stop=(kt == KT - 1))
                            tcs = rp.tile([128, QW], MMDT, tag="tcs")
                            nc.vector.tensor_tensor(
                                out=tcs[:], in0=pq[:],
                                in1=cs_sb[:, sb_i * QW:(sb_i + 1) * QW],
                                op=mybir.AluOpType.mult)
                            tsn = rp.tile([128, QW], MMDT, tag="tsn")
                            nc.vector.tensor_tensor(
                                out=tsn[:], in0=pq[:],
                                in1=sn_sb[:, sb_i * QW:(sb_i + 1) * QW],
                                op=mybir.AluOpType.mult)
                            pr = psR.tile([128, QW], f32, tag="rope")
                            nc.tensor.matmul(pr[:], rt2[:], tsn[:],
                                             start=True, stop=False)
                            nc.tensor.matmul(pr[:], ident[:], tcs[:],
                                             start=False, stop=True)
                            nc.scalar.activation(
                                dst[hp][:, sb_i * QW:(sb_i + 1) * QW],
                                pr[:], Copy)

                # ---- v projection (natural layout, ones-augmented) --------
                with tc.tile_pool(name="psV", bufs=2, space="PSUM") as psV:
                    for st in range(ST):
                        pv = psV.tile([128, GC], f32, tag="vproj")
                        for kt in range(KT):
                            nc.tensor.matmul(
                                pv[:],
                                xTr[kt][:, st * 128:(st + 1) * 128],
                                wv_s[:, kt * GC:(kt + 1) * GC],
                                start=(kt == 0), stop=(kt == KT - 1))
                        vt = v_sb[st]
                        nc.vector.tensor_copy(
                            vt[:].rearrange("p (h e) -> p h e", e=HD + 1)
                            [:, :, HD:HD + 1], ones_f[:])
                        nc.scalar.activation(
                            vt[:].rearrange("p (h e) -> p h e", e=HD + 1)
                            [:, :, 0:HD],
                            pv[:].rearrange("p (h d) -> p h d", d=HD),
                            Copy)

            # ---- phase B: attention, two heads interleaved ----------------
            # Heads 2*hp and 2*hp+1 sit at partitions 0-63 / 64-127 of one
            # qT/kT tile; their K=64 QK matmuls land on different PE
            # row-groups and overlap in the array.
            with tc.tile_pool(name="psS", bufs=2, space="PSUM") as psS, \
                 tc.tile_pool(name="psO", bufs=2, space="PSUM") as psO:
                for hp in range(2):
                    for qb in range(QB):
                        po = [psO.tile([HD + 1, QW], f32, tag=f"pvacc{s}",
                                       name=f"po_{hp}_{qb}_{s}")
                              for s in range(2)]
                        nsk = (qb + 1) * (QW // 128)
                        for kt in range(nsk):
                            prts = []
                            for s in range(2):   # head sub-index in pair
                                pb = s * 64
                                pscore = psS.tile([128, QW], f32,
                                                  tag=f"score{s}",
                                                  name=f"sc_{hp}_{qb}_{kt}_{s}")
                                nc.tensor.matmul(
                                    pscore[:],
                                    kTt[hp][pb:pb + 64,
                                            kt * 128:(kt + 1) * 128],
                                    qT[hp][pb:pb + 64,
                                           qb * QW:(qb + 1) * QW],
                                    start=True, stop=True)
                                prt = wp.tile([128, QW], MMDT,
                                              tag=f"probs{s}",
                                              name=f"pr_{hp}_{qb}_{kt}_{s}")
                                nc.scalar.activation(prt[:], pscore[:], Exp,
                                                     scale=0.125)
                                if kt >= nsk - 4:
                                    nc.gpsimd.affine_select(
                                        out=prt[:], in_=prt[:],
                                        pattern=[[1, QW]],
                                        base=qb * QW - kt * 128,
                                        channel_multiplier=-1,
                                        compare_op=mybir.AluOpType.is_ge,
                                        fill=0.0)
                                prts.append(prt)
                            for s in range(2):
                                h = 2 * hp + s
                                nc.tensor.matmul(
                                    po[s][:],
                                    v_sb[kt][:, h * (HD + 1):
                                             (h + 1) * (HD + 1)],
                                    prts[s][:],
                                    start=(kt == 0), stop=(kt == nsk - 1))
                        for s in range(2):
                            pb = s * 64
                            rc = sp.tile([1, QW], f32, tag="recip")
                            nc.vector.reciprocal(rc[:], po[s][HD:HD + 1, :])
                            bc = sp.tile([64, QW], f32, tag="bcast")
                            nc.gpsimd.partition_broadcast(bc[:], rc[:])
                            nc.vector.tensor_tensor(
                                out=attnT[hp][pb:pb + 64,
                                              qb * QW:(qb + 1) * QW],
                                in0=po[s][0:HD, :], in1=bc[:],
                                op=mybir.AluOpType.mult)

            # ---- phase C: output projection -------------------------------
            with tc.tile_pool(name="psC", bufs=3, space="PSUM") as psC:
                for st in range(ST):
                    for db in range(2):
                        pc = psC.tile([128, QW], f32, tag="oproj")
                        for kt in range(2):
                            nc.tensor.matmul(
                                pc[:],
                                attnT[kt][:, st * 128:(st + 1) * 128],
                                wo_s[kt][:, db * QW:(db + 1) * QW],
                                start=(kt == 0), stop=(kt == 1))
                        ob = op_.tile([128, QW], f32, tag="outsb")
                        nc.scalar.activation(ob[:], pc[:], Copy)
                        nc.sync.dma_start(
                            out[st * 128:(st + 1) * 128,
                                db * QW:(db + 1) * QW], ob[:])

    nc.compile()
    return nc


def _shard_inputs(x, cos, sin, wq, wk, wv, wo):
    x = np.ascontiguousarray(x, dtype=np.float32)
    cosT = np.ascontiguousarray(cos.reshape(S, HD).T, dtype=np.float32)
    sinT = np.ascontiguousarray(sin.reshape(S, HD).T, dtype=np.float32)
    cs2 = np.ascontiguousarray(np.concatenate([cosT, cosT], axis=0))
    sn2 = np.ascontiguousarray(np.concatenate([sinT, sinT], axis=0))
    in_maps = []
    for c in range(NCORES):
        b, g = c // GROUPS, c % GROUPS
        rows = slice(g * GC, (g + 1) * GC)
        in_maps.append({
            "xT": np.ascontiguousarray(x[b].T),
            "wqT": np.ascontiguousarray(np.asarray(wq, np.float32)[rows, :].T),
            "wkT": np.ascontiguousarray(np.asarray(wk, np.float32)[rows, :].T),
            "wvT": np.ascontiguousarray(np.asarray(wv, np.float32)[rows, :].T),
            "woT": np.ascontiguousarray(np.asarray(wo, np.float32)[:, rows].T),
            "cs2": cs2,
            "sn2": sn2,
        })
    return in_maps


def _run(inputs, trace=False, trace_kwargs=None):
    if "nc" not in _cache:
        _cache["nc"] = _build()
    nc = _cache["nc"]
    in_maps = _shard_inputs(
        inputs["x"], inputs["cos"], inputs["sin"],
        inputs["wq"], inputs["wk"], inputs["wv"], inputs["wo"])
    res = run_bass_kernel_spmd(
        nc, in_maps, list(range(NCORES)), trace=trace,
        **(trace_kwargs or {}))
    full = np.zeros((B, S, D), dtype=np.float32)
    for c in range(NCORES):
        full[c // GROUPS] += res.results[c]["out"]
    return full, res


def kernel(**inputs):
    full, _ = _run(inputs, trace=False)
    return full


# revision 11
# speedup vs baseline: 1.2795x; 1.1398x over previous
"""Trainium2 Bass kernel for nn_Attention_84473416778449.

Reference computation (B=2, S=2048, D=1024, H=16, HD=64, fp32):
    q/k/v = x @ w{q,k,v}.T ; RoPE(q, k) ; causal softmax attention ; out @ wo.T

Sharding: 8 cores = (batch 2) x (head-group 4). Each core computes 4 heads of
one batch end-to-end and a partial output projection over its 256 channels;
the host sums the 4 partials per batch.

On-chip layout (per core):
    xT    [1024, 2048]  (host pre-transposed x[b].T)
    wqT/wkT/wvT [1024, 256] (host pre-transposed w[rows,:].T)
    woT   [256, 1024]   (host pre-transposed wo[:, cols].T)
    qT/kT [2 x (128, 2048)] two heads per tile (dh on partitions), RoPE applied
    v     [16 x (128, 4*65)] s on partitions, per-head 64 cols + ones col
          (ones column makes the PV matmul also produce softmax denominators)
    scores^T tiles [sk=128, sq=512] -> exp on ACT -> causal mask via
          affine_select -> PV accumulate -> normalize by reciprocal of row 64.

Matmul operands are fp16 (full PE rate + fast weight load; ~5e-4 rounding,
4 extra mantissa bits vs bf16); accumulation is always fp32 in PSUM. Causal
masking is generated on-chip; fully-masked score tiles are skipped entirely.
Paired heads live at partitions 0-63 / 64-127 of one tile, so their K=64
QK matmuls target different PE row-groups and run concurrently.
"""
import sys

if "/opt/trn_rl_repo" not in sys.path:
    sys.path.insert(0, "/opt/trn_rl_repo")

import numpy as np

import concourse.bass as bass
import concourse.mybir as mybir
import concourse.tile as tile
from concourse import bacc
from concourse.bass_utils import run_bass_kernel_spmd

B, S, D, H, HD = 2, 2048, 1024, 16, 64
NCORES = 8
GROUPS = 4            # head groups
GH = H // GROUPS      # heads per group = 4
GC = GH * HD          # channels per group = 256
KT = D // 128         # 8 k-tiles over D
ST = S // 128         # 16 s-tiles
QB = 4                # sq blocks of 512
QW = S // QB          # 512

f32 = mybir.dt.float32
MMDT = mybir.dt.float16   # matmul-operand dtype
Exp = mybir.ActivationFunctionType.Exp
Copy = mybir.ActivationFunctionType.Copy

_cache = {}


def _build():
    nc = bacc.Bacc("TRN2", num_devices=NCORES)

    xT = nc.dram_tensor("xT", [D, S], f32, kind="ExternalInput").ap()
    wqT = nc.dram_tensor("wqT", [D, GC], f32, kind="ExternalInput").ap()
    wkT = nc.dram_tensor("wkT", [D, GC], f32, kind="ExternalInput").ap()
    wvT = nc.dram_tensor("wvT", [D, GC], f32, kind="ExternalInput").ap()
    woT = nc.dram_tensor("woT", [GC, D], f32, kind="ExternalInput").ap()
    cs2 = nc.dram_tensor("cs2", [128, S], f32, kind="ExternalInput").ap()
    sn2 = nc.dram_tensor("sn2", [128, S], f32, kind="ExternalInput").ap()
    out = nc.dram_tensor("out", [S, D], f32, kind="ExternalOutput").ap()

    with tile.TileContext(nc) as tc:
        with tc.tile_pool(name="persist", bufs=1) as pp, \
             tc.tile_pool(name="rope", bufs=3) as rp, \
             tc.tile_pool(name="probs", bufs=6) as wp, \
             tc.tile_pool(name="outsb", bufs=3) as op_, \
             tc.tile_pool(name="small", bufs=3) as sp:

            # ---- constants (built in fp32, cast via DVE copy) --------------
            cscratch = pp.tile([128, 128], f32, tag="cscratch")
            nc.gpsimd.memset(cscratch[:], 0.0)
            for blk in range(2):
                sub = cscratch[blk * 64:(blk + 1) * 64,
                               blk * 64:(blk + 1) * 64]
                # -1 where p - f == 32  (covers f<32 inside a 64x64 block)
                nc.gpsimd.affine_select(
                    out=sub, in_=sub, pattern=[[-1, 64]], base=-32,
                    channel_multiplier=1,
                    compare_op=mybir.AluOpType.not_equal, fill=-1.0)
                # +1 where f - p == 32
                nc.gpsimd.affine_select(
                    out=sub, in_=sub, pattern=[[1, 64]], base=-32,
                    channel_multiplier=-1,
                    compare_op=mybir.AluOpType.not_equal, fill=1.0)
            rt2 = pp.tile([128, 128], MMDT, tag="rt2")
            nc.vector.tensor_copy(rt2[:], cscratch[:])
            cscratch2 = pp.tile([128, 128], f32, tag="cscratch2")
            nc.gpsimd.memset(cscratch2[:], 0.0)
            nc.gpsimd.affine_select(
                out=cscratch2[:], in_=cscratch2[:], pattern=[[-1, 128]],
                base=0, channel_multiplier=1,
                compare_op=mybir.AluOpType.not_equal, fill=1.0)
            ident = pp.tile([128, 128], MMDT, tag="ident")
            nc.vector.tensor_copy(ident[:], cscratch2[:])
            ones_f = pp.tile([128, GH], f32, tag="ones_f")
            nc.gpsimd.memset(ones_f[:], 1.0)

            # ---- loads (gpsimd SWDGE casts fp32 -> fp16 in flight) ---------
            xTr = []
            for kt in range(KT):
                t = pp.tile([128, S], MMDT, tag=f"xT{kt}", name=f"xTr{kt}")
                nc.gpsimd.dma_start(t[:], xT[kt * 128:(kt + 1) * 128, :])
                xTr.append(t)

            def load_wT(src):
                t = pp.tile([128, KT * GC], MMDT, tag=f"w{src.tensor.name}",
                            name=f"w{src.tensor.name}")
                nc.gpsimd.dma_start(
                    t[:].rearrange("p (t c) -> p t c", c=GC),
                    src.rearrange("(t p) c -> p t c", p=128))
                return t

            wq_s, wk_s, wv_s = load_wT(wqT), load_wT(wkT), load_wT(wvT)
            wo_s = []
            for kt in range(2):
                t = pp.tile([128, D], MMDT, tag=f"wo{kt}", name=f"wo{kt}")
                nc.gpsimd.dma_start(t[:], woT[kt * 128:(kt + 1) * 128, :])
                wo_s.append(t)
            cs_sb = pp.tile([128, S], f32, tag="cs")
            nc.sync.dma_start(cs_sb[:], cs2[:])
            sn_sb = pp.tile([128, S], f32, tag="sn")
            nc.sync.dma_start(sn_sb[:], sn2[:])

            qT = [pp.tile([128, S], MMDT, tag=f"qT{i}", name=f"qT{i}")
                  for i in range(2)]
            kTt = [pp.tile([128, S], MMDT, tag=f"kT{i}", name=f"kT{i}")
                   for i in range(2)]
            attnT = [pp.tile([128, S], MMDT, tag=f"aT{i}", name=f"aT{i}")
                     for i in range(2)]
            v_sb = [pp.tile([128, GH * (HD + 1)], MMDT, tag=f"v{i}",
                            name=f"v{i}") for i in range(ST)]

            # ---- phase A: q/k projections + RoPE --------------------------
            with tc.tile_pool(name="psA", bufs=3, space="PSUM") as psA, \
                 tc.tile_pool(name="psR", bufs=2, space="PSUM") as psR:
                for w_src, dst in ((wq_s, qT), (wk_s, kTt)):
                    for hp in range(2):
                        for sb_i in range(QB):
                            pq = psA.tile([128, QW], f32, tag="proj")
                            for kt in range(KT):
                                nc.tensor.matmul(
                                    pq[:],
                                    w_src[:, kt * GC + hp * 128:
                                          kt * GC + hp * 128 + 128],
                                    xTr[kt][:, sb_i * QW:(sb_i + 1) * QW],
                                    start=(kt == 0), stop=(kt == KT - 1))
                            tcs = rp.tile([128, QW], MMDT, tag="tcs")
                            nc.vector.tensor_tensor(
                                out=tcs[:], in0=pq[:],
                                in1=cs_sb[:, sb_i * QW:(sb_i + 1) * QW],
                                op=mybir.AluOpType.mult)
                            tsn = rp.tile([128, QW], MMDT, tag="tsn")
                            nc.vector.tensor_tensor(
                                out=tsn[:], in0=pq[:],
                                in1=sn_sb[:, sb_i * QW:(sb_i + 1) * QW],
                                op=mybir.AluOpType.mult)
                            pr = psR.tile([128, QW], f32, tag="rope")
                            nc.tensor.matmul(pr[:], rt2[:], tsn[:],
                                             start=True, stop=False)
                            nc.tensor.matmul(pr[:], ident[:], tcs[:],
                                             start=False, stop=True)
                            nc.scalar.activation(
                                dst[hp][:, sb_i * QW:(sb_i + 1) * QW],
                                pr[:], Copy)

                # ---- v projection (natural layout, ones-augmented) --------
                with tc.tile_pool(name="psV", bufs=2, space="PSUM") as psV:
                    for st in range(ST):
                        pv = psV.tile([128, GC], f32, tag="vproj")
                        for kt in range(KT):
                            nc.tensor.matmul(
                                pv[:],
                                xTr[kt][:, st * 128:(st + 1) * 128],
                                wv_s[:, kt * GC:(kt + 1) * GC],
                                start=(kt == 0), stop=(kt == KT - 1))
                        vt = v_sb[st]
                        nc.vector.tensor_copy(
                            vt[:].rearrange("p (h e) -> p h e", e=HD + 1)
                            [:, :, HD:HD + 1], ones_f[:])
                        nc.scalar.activation(
                            vt[:].rearrange("p (h e) -> p h e", e=HD + 1)
                            [:, :, 0:HD],
                            pv[:].rearrange("p (h d) -> p h d", d=HD),
                            Copy)

            # ---- phase B: attention, two heads interleaved ----------------
            # Heads 2*hp and 2*hp+1 sit at partitions 0-63 / 64-127 of one
            # qT/kT tile; their K=64 QK matmuls land on different PE
            # row-groups and overlap in the array.
            # HAM does not count the K=64 / M=65 attention matmuls as PE
            # activity, so the clock gate drops to 1.2 GHz; a full 128x128
            # "warmer" matmul every few steps keeps the PE at 2.4 GHz.
            with tc.tile_pool(name="psS", bufs=2, space="PSUM") as psS, \
                 tc.tile_pool(name="psO", bufs=1, space="PSUM") as psO, \
                 tc.tile_pool(name="psW", bufs=1, space="PSUM") as psW:
                warm_ctr = 0
                for hp in range(2):
                    for qb in range(QB):
                        po = [psO.tile([HD + 1, QW], f32, tag=f"pvacc{s}",
                                       name=f"po_{hp}_{qb}_{s}")
                              for s in range(2)]
                        nsk = (qb + 1) * (QW // 128)
                        for kt in range(nsk):
                            warm_ctr += 1
                            if warm_ctr % 3 == 0:
                                pw = psW.tile([128, QW], f32, tag="warm")
                                nc.tensor.matmul(pw[:], ident[:],
                                                 xTr[0][:, 0:QW],
                                                 start=True, stop=True,
                                                 skip_group_check=True)
                            prts = []
                            for s in range(2):   # head sub-index in pair
                                pb = s * 64
                                pscore = psS.tile([128, QW], f32,
                                                  tag=f"score{s}",
                                                  name=f"sc_{hp}_{qb}_{kt}_{s}")
                                nc.tensor.matmul(
                                    pscore[:],
                                    kTt[hp][pb:pb + 64,
                                            kt * 128:(kt + 1) * 128],
                                    qT[hp][pb:pb + 64,
                                           qb * QW:(qb + 1) * QW],
                                    start=True, stop=True)
                                prt = wp.tile([128, QW], MMDT,
                                              tag=f"probs{s}",
                                              name=f"pr_{hp}_{qb}_{kt}_{s}")
                                nc.scalar.activation(prt[:], pscore[:], Exp,
                                                     scale=0.125)
                                if kt >= nsk - 4:
                                    nc.gpsimd.affine_select(
                                        out=prt[:], in_=prt[:],
                                        pattern=[[1, QW]],
                                        base=qb * QW - kt * 128,
                                        channel_multiplier=-1,
                                        compare_op=mybir.AluOpType.is_ge,
                                        fill=0.0)
                                prts.append(prt)
                            for s in range(2):
                                h = 2 * hp + s
                                nc.tensor.matmul(
                                    po[s][:],
                                    v_sb[kt][:, h * (HD + 1):
                                             (h + 1) * (HD + 1)],
                                    prts[s][:],
                                    start=(kt == 0), stop=(kt == nsk - 1))
                        for s in range(2):
                            pb = s * 64
                            d0 = sp.tile([1, QW], f32, tag="den0")
                            nc.vector.tensor_copy(d0[:], po[s][HD:HD + 1, :])
                            dn = sp.tile([64, QW], f32, tag="denb")
                            nc.gpsimd.partition_broadcast(dn[:], d0[:])
                            rcb = sp.tile([64, QW], f32, tag="recb")
                            scr = sp.tile([64, QW], f32, tag="scrb")
                            nc.vector.reciprocal_approx_accurate(
                                out=rcb[:], in_=dn[:], scratch=scr[:])
                            nc.vector.tensor_tensor(
                                out=attnT[hp][pb:pb + 64,
                                              qb * QW:(qb + 1) * QW],
                                in0=po[s][0:HD, :], in1=rcb[:],
                                op=mybir.AluOpType.mult)

            # ---- phase C: output projection -------------------------------
            with tc.tile_pool(name="psC", bufs=3, space="PSUM") as psC:
                for st in range(ST):
                    for db in range(2):
                        pc = psC.tile([128, QW], f32, tag="oproj")
                        for kt in range(2):
                            nc.tensor.matmul(
                                pc[:],
                                attnT[kt][:, st * 128:(st + 1) * 128],
                                wo_s[kt][:, db * QW:(db + 1) * QW],
                                start=(kt == 0), stop=(kt == 1))
                        ob = op_.tile([128, QW], f32, tag="outsb")
                        nc.vector.tensor_copy(ob[:], pc[:])
                        nc.sync.dma_start(
                            out[st * 128:(st + 1) * 128,
                                db * QW:(db + 1) * QW], ob[:])

    nc.compile()
    return nc


def _shard_inputs(x, cos, sin, wq, wk, wv, wo):
    x = np.ascontiguousarray(x, dtype=np.float32)
    cosT = np.ascontiguousarray(cos.reshape(S, HD).T, dtype=np.float32)
    sinT = np.ascontiguousarray(sin.reshape(S, HD).T, dtype=np.float32)
    cs2 = np.ascontiguousarray(np.concatenate([cosT, cosT], axis=0))
    sn2 = np.ascontiguousarray(np.concatenate([sinT, sinT], axis=0))
    in_maps = []
    for c in range(NCORES):
        b, g = c // GROUPS, c % GROUPS
        rows = slice(g * GC, (g + 1) * GC)
        in_maps.append({
            "xT": np.ascontiguousarray(x[b].T),
            "wqT": np.ascontiguousarray(np.asarray(wq, np.float32)[rows, :].T),
            "wkT": np.ascontiguousarray(np.asarray(wk, np.float32)[rows, :].T),
            "wvT": np.ascontiguousarray(np.asarray(wv, np.float32)[rows, :].T),
            "woT": np.ascontiguousarray(np.asarray(wo, np.float32)[:, rows].T),
            "cs2": cs2,
            "sn2": sn2,
        })
    return in_maps


def _run(inputs, trace=False, trace_kwargs=None):
    if "nc" not in _cache:
        _cache["nc"] = _build()
    nc = _cache["nc"]
    in_maps = _shard_inputs(
        inputs["x"], inputs["cos"], inputs["sin"],
        inputs["wq"], inputs["wk"], inputs["wv"], inputs["wo"])
    res = run_bass_kernel_spmd(
        nc, in_maps, list(range(NCORES)), trace=trace,
        **(trace_kwargs or {}))
    full = np.zeros((B, S, D), dtype=np.float32)
    for c in range(NCORES):
        full[c // GROUPS] += res.results[c]["out"]
    return full, res


def kernel(**inputs):
    full, _ = _run(inputs, trace=False)
    return full


# revision 13
# speedup vs baseline: 1.3897x; 1.0861x over previous
"""Trainium2 Bass kernel for nn_Attention_84473416778449.

Reference computation (B=2, S=2048, D=1024, H=16, HD=64, fp32):
    q/k/v = x @ w{q,k,v}.T ; RoPE(q, k) ; causal softmax attention ; out @ wo.T

Sharding: 8 cores = (batch 2) x (head-group 4). Each core computes 4 heads of
one batch end-to-end and a partial output projection over its 256 channels;
the host sums the 4 partials per batch.

On-chip layout (per core):
    xT    [1024, 2048]  (host pre-transposed x[b].T)
    wqT/wkT/wvT [1024, 256] (host pre-transposed w[rows,:].T)
    woT   [256, 1024]   (host pre-transposed wo[:, cols].T)
    qT/kT [2 x (128, 2048)] two heads per tile (dh on partitions), RoPE applied
    v     [16 x (128, 4*65)] s on partitions, per-head 64 cols + ones col
          (ones column makes the PV matmul also produce softmax denominators)
    scores^T tiles [sk=128, sq=512] -> exp on ACT -> causal mask via
          affine_select -> PV accumulate -> normalize by reciprocal of row 64.

Matmul operands are fp16 (full PE rate + fast weight load; ~5e-4 rounding,
4 extra mantissa bits vs bf16); accumulation is always fp32 in PSUM. Causal
masking is generated on-chip; fully-masked score tiles are skipped entirely.
Paired heads live at partitions 0-63 / 64-127 of one tile, so their K=64
QK matmuls target different PE row-groups and run concurrently.
"""
import sys

if "/opt/trn_rl_repo" not in sys.path:
    sys.path.insert(0, "/opt/trn_rl_repo")

import numpy as np

import concourse.bass as bass
import concourse.mybir as mybir
import concourse.tile as tile
from concourse import bacc
from concourse.bass_utils import run_bass_kernel_spmd

B, S, D, H, HD = 2, 2048, 1024, 16, 64
NCORES = 8
GROUPS = 4            # head groups
GH = H // GROUPS      # heads per group = 4
GC = GH * HD          # channels per group = 256
KT = D // 128         # 8 k-tiles over D
ST = S // 128         # 16 s-tiles
QB = 4                # sq blocks of 512
QW = S // QB          # 512

f32 = mybir.dt.float32
MMDT = mybir.dt.float16   # matmul-operand dtype
Exp = mybir.ActivationFunctionType.Exp
Copy = mybir.ActivationFunctionType.Copy

_cache = {}


def _build():
    nc = bacc.Bacc("TRN2", num_devices=NCORES)

    xT = nc.dram_tensor("xT", [D, S], f32, kind="ExternalInput").ap()
    wqT = nc.dram_tensor("wqT", [D, GC], f32, kind="ExternalInput").ap()
    wkT = nc.dram_tensor("wkT", [D, GC], f32, kind="ExternalInput").ap()
    wvT = nc.dram_tensor("wvT", [D, GC], f32, kind="ExternalInput").ap()
    woT = nc.dram_tensor("woT", [GC, D], f32, kind="ExternalInput").ap()
    cs2 = nc.dram_tensor("cs2", [128, S], f32, kind="ExternalInput").ap()
    sn2 = nc.dram_tensor("sn2", [128, S], f32, kind="ExternalInput").ap()
    out = nc.dram_tensor("out", [S, D], f32, kind="ExternalOutput").ap()

    with tile.TileContext(nc) as tc:
        with tc.tile_pool(name="persist", bufs=1) as pp, \
             tc.tile_pool(name="rope", bufs=3) as rp, \
             tc.tile_pool(name="probs", bufs=6) as wp, \
             tc.tile_pool(name="outsb", bufs=3) as op_, \
             tc.tile_pool(name="small", bufs=3) as sp:

            # ---- constants (built in fp32, cast via DVE copy) --------------
            cscratch = pp.tile([128, 128], f32, tag="cscratch")
            nc.gpsimd.memset(cscratch[:], 0.0)
            for blk in range(2):
                sub = cscratch[blk * 64:(blk + 1) * 64,
                               blk * 64:(blk + 1) * 64]
                # -1 where p - f == 32  (covers f<32 inside a 64x64 block)
                nc.gpsimd.affine_select(
                    out=sub, in_=sub, pattern=[[-1, 64]], base=-32,
                    channel_multiplier=1,
                    compare_op=mybir.AluOpType.not_equal, fill=-1.0)
                # +1 where f - p == 32
                nc.gpsimd.affine_select(
                    out=sub, in_=sub, pattern=[[1, 64]], base=-32,
                    channel_multiplier=-1,
                    compare_op=mybir.AluOpType.not_equal, fill=1.0)
            rt2 = pp.tile([128, 128], MMDT, tag="rt2")
            nc.vector.tensor_copy(rt2[:], cscratch[:])
            cscratch2 = pp.tile([128, 128], f32, tag="cscratch2")
            nc.gpsimd.memset(cscratch2[:], 0.0)
            nc.gpsimd.affine_select(
                out=cscratch2[:], in_=cscratch2[:], pattern=[[-1, 128]],
                base=0, channel_multiplier=1,
                compare_op=mybir.AluOpType.not_equal, fill=1.0)
            ident = pp.tile([128, 128], MMDT, tag="ident")
            nc.vector.tensor_copy(ident[:], cscratch2[:])
            ones_f = pp.tile([128, GH], f32, tag="ones_f")
            nc.gpsimd.memset(ones_f[:], 1.0)

            # ---- loads (gpsimd SWDGE casts fp32 -> fp16 in flight) ---------
            xTr = []
            for kt in range(KT):
                t = pp.tile([128, S], MMDT, tag=f"xT{kt}", name=f"xTr{kt}")
                nc.gpsimd.dma_start(t[:], xT[kt * 128:(kt + 1) * 128, :])
                xTr.append(t)

            def load_wT(src):
                t = pp.tile([128, KT * GC], MMDT, tag=f"w{src.tensor.name}",
                            name=f"w{src.tensor.name}")
                nc.gpsimd.dma_start(
                    t[:].rearrange("p (t c) -> p t c", c=GC),
                    src.rearrange("(t p) c -> p t c", p=128))
                return t

            wq_s, wk_s, wv_s = load_wT(wqT), load_wT(wkT), load_wT(wvT)
            wo_s = []
            for kt in range(2):
                t = pp.tile([128, D], MMDT, tag=f"wo{kt}", name=f"wo{kt}")
                nc.gpsimd.dma_start(t[:], woT[kt * 128:(kt + 1) * 128, :])
                wo_s.append(t)
            cs_sb = pp.tile([128, S], f32, tag="cs")
            nc.sync.dma_start(cs_sb[:], cs2[:])
            sn_sb = pp.tile([128, S], f32, tag="sn")
            nc.sync.dma_start(sn_sb[:], sn2[:])

            qT = [pp.tile([128, S], MMDT, tag=f"qT{i}", name=f"qT{i}")
                  for i in range(2)]
            kTt = [pp.tile([128, S], MMDT, tag=f"kT{i}", name=f"kT{i}")
                   for i in range(2)]
            attnT = [pp.tile([128, S], MMDT, tag=f"aT{i}", name=f"aT{i}")
                     for i in range(2)]
            v_sb = [pp.tile([128, GH * (HD + 1)], MMDT, tag=f"v{i}",
                            name=f"v{i}") for i in range(ST)]

            # ---- phase A: q/k projections + RoPE --------------------------
            with tc.tile_pool(name="psA", bufs=3, space="PSUM") as psA, \
                 tc.tile_pool(name="psR", bufs=2, space="PSUM") as psR:
                for w_src, dst in ((wq_s, qT), (wk_s, kTt)):
                    for hp in range(2):
                        for sb_i in range(QB):
                            pq = psA.tile([128, QW], f32, tag="proj")
                            for kt in range(KT):
                                nc.tensor.matmul(
                                    pq[:],
                                    w_src[:, kt * GC + hp * 128:
                                          kt * GC + hp * 128 + 128],
                                    xTr[kt][:, sb_i * QW:(sb_i + 1) * QW],
                                    start=(kt == 0), stop=(kt == KT - 1))
                            tcs = rp.tile([128, QW], MMDT, tag="tcs")
                            nc.vector.tensor_tensor(
                                out=tcs[:], in0=pq[:],
                                in1=cs_sb[:, sb_i * QW:(sb_i + 1) * QW],
                                op=mybir.AluOpType.mult)
                            tsn = rp.tile([128, QW], MMDT, tag="tsn")
                            nc.vector.tensor_tensor(
                                out=tsn[:], in0=pq[:],
                                in1=sn_sb[:, sb_i * QW:(sb_i + 1) * QW],
                                op=mybir.AluOpType.mult)
                            pr = psR.tile([128, QW], f32, tag="rope")
                            nc.tensor.matmul(pr[:], rt2[:], tsn[:],
                                             start=True, stop=False)
                            nc.tensor.matmul(pr[:], ident[:], tcs[:],
                                             start=False, stop=True)
                            nc.vector.tensor_copy(
                                dst[hp][:, sb_i * QW:(sb_i + 1) * QW],
                                pr[:])

                # ---- v projection (natural layout, ones-augmented) --------
                with tc.tile_pool(name="psV", bufs=2, space="PSUM") as psV:
                    for st in range(ST):
                        pv = psV.tile([128, GC], f32, tag="vproj")
                        for kt in range(KT):
                            nc.tensor.matmul(
                                pv[:],
                                xTr[kt][:, st * 128:(st + 1) * 128],
                                wv_s[:, kt * GC:(kt + 1) * GC],
                                start=(kt == 0), stop=(kt == KT - 1))
                        vt = v_sb[st]
                        nc.vector.tensor_copy(
                            vt[:].rearrange("p (h e) -> p h e", e=HD + 1)
                            [:, :, HD:HD + 1], ones_f[:])
                        nc.scalar.activation(
                            vt[:].rearrange("p (h e) -> p h e", e=HD + 1)
                            [:, :, 0:HD],
                            pv[:].rearrange("p (h d) -> p h d", d=HD),
                            Copy)

            # ---- phase B: attention, two heads interleaved ----------------
            # Heads 2*hp and 2*hp+1 sit at partitions 0-63 / 64-127 of one
            # qT/kT tile; their K=64 QK matmuls land on different PE
            # row-groups and overlap in the array.
            # HAM does not count the K=64 / M=65 attention matmuls as PE
            # activity, so the clock gate drops to 1.2 GHz; a full 128x128
            # "warmer" matmul every few steps keeps the PE at 2.4 GHz.
            with tc.tile_pool(name="psS", bufs=2, space="PSUM") as psS, \
                 tc.tile_pool(name="psO", bufs=1, space="PSUM") as psO, \
                 tc.tile_pool(name="psW", bufs=1, space="PSUM") as psW, \
                 tc.tile_pool(name="psC", bufs=1, space="PSUM") as psC:
                warm_ctr = 0
                for qb in range(QB):
                    for hp in range(2):
                        po = [psO.tile([HD + 1, QW], f32, tag=f"pvacc{s}",
                                       name=f"po_{hp}_{qb}_{s}")
                              for s in range(2)]
                        nsk = (qb + 1) * (QW // 128)
                        for kt in range(nsk):
                            warm_ctr += 1
                            if warm_ctr % 2 == 0:
                                pw = psW.tile([128, QW], f32, tag="warm")
                                nc.tensor.matmul(pw[:], ident[:],
                                                 xTr[0][:, 0:QW],
                                                 start=True, stop=True,
                                                 skip_group_check=True)
                            prts = []
                            for s in range(2):   # head sub-index in pair
                                pb = s * 64
                                pscore = psS.tile([128, QW], f32,
                                                  tag=f"score{s}",
                                                  name=f"sc_{hp}_{qb}_{kt}_{s}")
                                nc.tensor.matmul(
                                    pscore[:],
                                    kTt[hp][pb:pb + 64,
                                            kt * 128:(kt + 1) * 128],
                                    qT[hp][pb:pb + 64,
                                           qb * QW:(qb + 1) * QW],
                                    start=True, stop=True,
                                    tile_position=(pb, 0))
                                prt = wp.tile([128, QW], MMDT,
                                              tag=f"probs{s}",
                                              name=f"pr_{hp}_{qb}_{kt}_{s}")
                                nc.scalar.activation(prt[:], pscore[:], Exp,
                                                     scale=0.125)
                                if kt >= nsk - 4:
                                    nc.gpsimd.affine_select(
                                        out=prt[:], in_=prt[:],
                                        pattern=[[1, QW]],
                                        base=qb * QW - kt * 128,
                                        channel_multiplier=-1,
                                        compare_op=mybir.AluOpType.is_ge,
                                        fill=0.0)
                                prts.append(prt)
                            for s in range(2):
                                h = 2 * hp + s
                                nc.tensor.matmul(
                                    po[s][:],
                                    v_sb[kt][:, h * (HD + 1):
                                             (h + 1) * (HD + 1)],
                                    prts[s][:],
                                    start=(kt == 0), stop=(kt == nsk - 1))
                        for s in range(2):
                            pb = s * 64
                            d0 = sp.tile([1, QW], f32, tag="den0")
                            nc.vector.tensor_copy(d0[:], po[s][HD:HD + 1, :])
                            dn = sp.tile([64, QW], f32, tag="denb")
                            nc.gpsimd.partition_broadcast(dn[:], d0[:])
                            rcb = sp.tile([64, QW], f32, tag="recb")
                            scr = sp.tile([64, QW], f32, tag="scrb")
                            nc.vector.reciprocal_approx_accurate(
                                out=rcb[:], in_=dn[:], scratch=scr[:])
                            nc.vector.tensor_tensor(
                                out=attnT[hp][pb:pb + 64,
                                              qb * QW:(qb + 1) * QW],
                                in0=po[s][0:HD, :], in1=rcb[:],
                                op=mybir.AluOpType.mult)

                    # ---- output projection for this sq block --------------
                    for st in range(qb * 4, (qb + 1) * 4):
                        for db in range(2):
                            pc = psC.tile([128, QW], f32, tag="oproj")
                            for kt in range(2):
                                nc.tensor.matmul(
                                    pc[:],
                                    attnT[kt][:, st * 128:(st + 1) * 128],
                                    wo_s[kt][:, db * QW:(db + 1) * QW],
                                    start=(kt == 0), stop=(kt == 1))
                            ob = op_.tile([128, QW], f32, tag="outsb")
                            nc.vector.tensor_copy(ob[:], pc[:])
                            nc.sync.dma_start(
                                out[st * 128:(st + 1) * 128,
                                    db * QW:(db + 1) * QW], ob[:])

    nc.compile()
    return nc


def _shard_inputs(x, cos, sin, wq, wk, wv, wo):
    x = np.ascontiguousarray(x, dtype=np.float32)
    cosT = np.ascontiguousarray(cos.reshape(S, HD).T, dtype=np.float32)
    sinT = np.ascontiguousarray(sin.reshape(S, HD).T, dtype=np.float32)
    cs2 = np.ascontiguousarray(np.concatenate([cosT, cosT], axis=0))
    sn2 = np.ascontiguousarray(np.concatenate([sinT, sinT], axis=0))
    in_maps = []
    for c in range(NCORES):
        b, g = c // GROUPS, c % GROUPS
        rows = slice(g * GC, (g + 1) * GC)
        in_maps.append({
            "xT": np.ascontiguousarray(x[b].T),
            "wqT": np.ascontiguousarray(np.asarray(wq, np.float32)[rows, :].T),
            "wkT": np.ascontiguousarray(np.asarray(wk, np.float32)[rows, :].T),
            "wvT": np.ascontiguousarray(np.asarray(wv, np.float32)[rows, :].T),
            "woT": np.ascontiguousarray(np.asarray(wo, np.float32)[:, rows].T),
            "cs2": cs2,
            "sn2": sn2,
        })
    return in_maps


def _run(inputs, trace=False, trace_kwargs=None):
    if "nc" not in _cache:
        _cache["nc"] = _build()
    nc = _cache["nc"]
    in_maps = _shard_inputs(
        inputs["x"], inputs["cos"], inputs["sin"],
        inputs["wq"], inputs["wk"], inputs["wv"], inputs["wo"])
    res = run_bass_kernel_spmd(
        nc, in_maps, list(range(NCORES)), trace=trace,
        **(trace_kwargs or {}))
    full = np.zeros((B, S, D), dtype=np.float32)
    for c in range(NCORES):
        full[c // GROUPS] += res.results[c]["out"]
    return full, res


def kernel(**inputs):
    full, _ = _run(inputs, trace=False)
    return full
